# revision 9
# baseline (speedup 1.0000x reference)
"""CBOW word2vec negative-sampling loss on 8 Trainium2 NeuronCores.

Strategy (data-parallel over batch):
  - batch B=16384 split into 8 shards of 2048 samples (one per core)
  - u_weight/v_weight concatenated host-side into one [200000, 128] table
    (replicated per core); all 21 embedding-row reads per sample
    (10 ctx + 1 pos + 10 neg, v-rows offset by VOCAB) are indirect DMA
    gathers of 128 rows each ([128,1] offset APs — the only offset shape
    this toolchain generates correct descriptors for), 21 per 128-sample
    block
  - per block on-chip: sum ctx rows (DVE reduce), 11 fused dot products
    (scalar_tensor_tensor with accum_out), clip, softplus = Ln(1+Exp(x))
    on ACT with fused free-dim accumulation into the accumulator column
  - per-core partial sums [128, 16] are summed + averaged on host
"""

import numpy as np

VOCAB = 100000
DIM = 128
B = 16384
CTX = 10
NNEG = 10
N_CORES = 8
P = 128
B_SHARD = B // N_CORES          # 2048
NBLK = B_SHARD // P             # 16
K = CTX + 1 + NNEG              # 21 gathered rows per sample


def _split_excess_waits(nc, mybir, max_waits=1):
    """This walrus build rejects instructions carrying more than ~1 sync
    wait (Tile's kernel-tail drain can carry several). Hoist excess waits
    into standalone nops right before the offending instruction — same
    engine, so the in-order stream gives identical semantics."""
    n_split = 0
    for func in nc.m.functions:
        for bb in func.blocks:
            out = []
            changed = False
            for inst in bb.instructions:
                si = inst.sync_info
                if si is not None and len(si.on_wait) > max_waits:
                    waits = list(si.on_wait)
                    for k, w in enumerate(waits[:-max_waits]):
                        nop = mybir.InstNoOp(
                            name=f"wsplit_{inst.name}_{k}", ins=[], outs=[]
                        )
                        nop.engine = inst.engine
                        nop.sync_info = mybir.SyncInfo(on_wait=[w], on_update=[])
                        nc.register_instruction(nop)
                        out.append(nop)
                        n_split += 1
                    inst.sync_info = mybir.SyncInfo(
                        on_wait=waits[-max_waits:], on_update=si.on_update
                    )
                    changed = True
                out.append(inst)
            if changed:
                bb.instructions = out
    return n_split


_PROGRAM_CACHE = {}


def _build_program(gather_bufs=6):
    if gather_bufs in _PROGRAM_CACHE:
        return _PROGRAM_CACHE[gather_bufs]

    import concourse.bass as bass
    import concourse.tile as tile
    import concourse.mybir as mybir

    f32 = mybir.dt.float32
    i32 = mybir.dt.int32
    ND = K - CTX  # 11 dot products per sample (1 pos + 10 neg)

    nc = bass.Bass()
    table = nc.dram_tensor("table", [2 * VOCAB, DIM], f32, kind="ExternalInput")
    idx = nc.dram_tensor("idx", [P, NBLK * K], i32, kind="ExternalInput")
    out = nc.dram_tensor("out", [P, NBLK], f32, kind="ExternalOutput")

    with tile.TileContext(nc) as tc:
        with (
            tc.tile_pool(name="const", bufs=1) as cpool,
            tc.tile_pool(name="gather", bufs=gather_bufs) as gpool,
            tc.tile_pool(name="small", bufs=4) as spool,
            tc.tile_pool(name="scratch", bufs=4) as scpool,
        ):
            # block-0 columns first so the first gathers start immediately,
            # then the rest of the index table loads in parallel
            idx0 = cpool.tile([P, K], i32)
            nc.sync.dma_start(idx0[:], idx[:, 0:K])
            idx_t = cpool.tile([P, NBLK * K], i32)
            nc.sync.dma_start(idx_t[:, K:], idx[:, K:])
            acc = cpool.tile([P, NBLK], f32)

            for j in range(NBLK):
                g = gpool.tile([P, K, DIM], f32, tag="g")
                # One [128,1]-offset gather per role: the only offset-AP
                # shape this walrus generates correct descriptors for.
                for k in range(K):
                    nc.gpsimd.indirect_dma_start(
                        out=g[:, k, :],
                        out_offset=None,
                        in_=table[:],
                        in_offset=bass.IndirectOffsetOnAxis(
                            ap=(idx0 if j == 0 else idx_t)[
                                :, (j * K + k if j else k) : (j * K + k if j else k) + 1
                            ],
                            axis=0,
                        ),
                    )

                # sum of the 10 context rows -> [P, DIM]
                su = spool.tile([P, DIM], f32, tag="su")
                nc.vector.tensor_reduce(
                    out=su[:],
                    in_=g[:, 0:CTX, :].rearrange("p n d -> p d n"),
                    axis=mybir.AxisListType.X,
                    op=mybir.AluOpType.add,
                )

                # 11 fused dots: raw[:, n] = sum_d (±0.1 * v_row_n) * su
                # n=0 (pos sample) carries the minus sign so that the loss is
                # softplus(raw_n) uniformly for all n.
                raw = spool.tile([P, ND], f32, tag="raw")
                for n in range(ND):
                    so = scpool.tile([P, DIM], f32, tag="so")
                    nc.vector.scalar_tensor_tensor(
                        out=so[:],
                        in0=g[:, CTX + n, :],
                        scalar=(-1.0 if n == 0 else 1.0) / CTX,
                        in1=su[:],
                        op0=mybir.AluOpType.mult,
                        op1=mybir.AluOpType.mult,
                        accum_out=raw[:, n : n + 1],
                    )

                # clip to [-10, 10] in one fused op
                rc = spool.tile([P, ND], f32, tag="rc")
                nc.vector.tensor_scalar(
                    out=rc[:],
                    in0=raw[:],
                    scalar1=-10.0,
                    scalar2=10.0,
                    op0=mybir.AluOpType.max,
                    op1=mybir.AluOpType.min,
                )

                # softplus(x) = ln(1 + exp(x)); accumulate the 11 terms into
                # this block's accumulator column.
                ex = scpool.tile([P, ND], f32, tag="ex")
                nc.scalar.activation(
                    out=ex[:],
                    in_=rc[:],
                    func=mybir.ActivationFunctionType.Exp,
                )
                sp = scpool.tile([P, ND], f32, tag="sp")
                nc.scalar.activation(
                    out=sp[:],
                    in_=ex[:],
                    func=mybir.ActivationFunctionType.Ln,
                    bias=1.0,
                    accum_out=acc[:, j : j + 1],
                )

            nc.sync.dma_start(out[:], acc[:])

    _split_excess_waits(nc, mybir)
    _PROGRAM_CACHE[gather_bufs] = nc
    return nc


def _prep_inputs(pos_u, pos_v, neg_v, u_weight, v_weight):
    """Shard + repack host-side. Returns per-core input maps."""
    table = np.ascontiguousarray(
        np.concatenate(
            [np.asarray(u_weight, np.float32), np.asarray(v_weight, np.float32)],
            axis=0,
        )
    )
    pos_u = np.asarray(pos_u, np.int32)
    pos_v = np.asarray(pos_v, np.int32)
    neg_v = np.asarray(neg_v, np.int32)

    in_maps = []
    for c in range(N_CORES):
        s = slice(c * B_SHARD, (c + 1) * B_SHARD)
        ia = np.empty((B_SHARD, K), np.int32)
        ia[:, 0:CTX] = pos_u[s]
        ia[:, CTX] = pos_v[s] + VOCAB
        ia[:, CTX + 1 : K] = neg_v[s] + VOCAB
        idx_dram = np.ascontiguousarray(
            ia.reshape(NBLK, P, K).transpose(1, 0, 2).reshape(P, NBLK * K)
        )
        in_maps.append({"table": table, "idx": idx_dram})
    return in_maps


def _run(pos_u, pos_v, neg_v, u_weight, v_weight, trace=False):
    from concourse.bass_utils import run_bass_kernel_spmd

    nc = _build_program()
    in_maps = _prep_inputs(pos_u, pos_v, neg_v, u_weight, v_weight)
    res = run_bass_kernel_spmd(nc, in_maps, list(range(N_CORES)), trace=trace)
    total = 0.0
    for c in range(N_CORES):
        total += res.results[c]["out"].sum(dtype=np.float64)
    loss = np.array(total / B, dtype=np.float32)
    return loss, res


def kernel(pos_u, pos_v, neg_v, u_weight, v_weight):
    loss, _ = _run(pos_u, pos_v, neg_v, u_weight, v_weight, trace=False)
    return loss


# revision 10
# speedup vs baseline: 1.0053x; 1.0053x over previous
"""CBOW word2vec negative-sampling loss on 8 Trainium2 NeuronCores.

Strategy (data-parallel over batch):
  - batch B=16384 split into 8 shards of 2048 samples (one per core)
  - u_weight/v_weight concatenated host-side into one [200000, 128] table
    (replicated per core); all 21 embedding-row reads per sample
    (10 ctx + 1 pos + 10 neg, v-rows offset by VOCAB) are indirect DMA
    gathers of 128 rows each ([128,1] offset APs — the only offset shape
    this toolchain generates correct descriptors for), 21 per 128-sample
    block
  - per block on-chip: sum ctx rows (DVE reduce), 11 fused dot products
    (scalar_tensor_tensor with accum_out), clip, softplus = Ln(1+Exp(x))
    on ACT with fused free-dim accumulation into the accumulator column
  - per-core partial sums [128, 16] are summed + averaged on host
"""

import numpy as np

VOCAB = 100000
DIM = 128
B = 16384
CTX = 10
NNEG = 10
N_CORES = 8
P = 128
B_SHARD = B // N_CORES          # 2048
NBLK = B_SHARD // P             # 16
K = CTX + 1 + NNEG              # 21 gathered rows per sample


def _split_excess_waits(nc, mybir, max_waits=1):
    """This walrus build rejects instructions carrying more than ~1 sync
    wait (Tile's kernel-tail drain can carry several). Hoist excess waits
    into standalone nops right before the offending instruction — same
    engine, so the in-order stream gives identical semantics."""
    n_split = 0
    for func in nc.m.functions:
        for bb in func.blocks:
            out = []
            changed = False
            for inst in bb.instructions:
                si = inst.sync_info
                if si is not None and len(si.on_wait) > max_waits:
                    waits = list(si.on_wait)
                    for k, w in enumerate(waits[:-max_waits]):
                        nop = mybir.InstNoOp(
                            name=f"wsplit_{inst.name}_{k}", ins=[], outs=[]
                        )
                        nop.engine = inst.engine
                        nop.sync_info = mybir.SyncInfo(on_wait=[w], on_update=[])
                        nc.register_instruction(nop)
                        out.append(nop)
                        n_split += 1
                    inst.sync_info = mybir.SyncInfo(
                        on_wait=waits[-max_waits:], on_update=si.on_update
                    )
                    changed = True
                out.append(inst)
            if changed:
                bb.instructions = out
    return n_split


_PROGRAM_CACHE = {}


def _build_program(gather_bufs=6):
    if gather_bufs in _PROGRAM_CACHE:
        return _PROGRAM_CACHE[gather_bufs]

    import concourse.bass as bass
    import concourse.tile as tile
    import concourse.mybir as mybir

    f32 = mybir.dt.float32
    i32 = mybir.dt.int32
    ND = K - CTX  # 11 dot products per sample (1 pos + 10 neg)

    nc = bass.Bass()
    table = nc.dram_tensor("table", [2 * VOCAB, DIM], f32, kind="ExternalInput")
    idx = nc.dram_tensor("idx", [P, NBLK * K], i32, kind="ExternalInput")
    out = nc.dram_tensor("out", [P, NBLK], f32, kind="ExternalOutput")

    with tile.TileContext(nc) as tc:
        with (
            tc.tile_pool(name="const", bufs=1) as cpool,
            tc.tile_pool(name="gather", bufs=gather_bufs) as gpool,
            tc.tile_pool(name="small", bufs=4) as spool,
            tc.tile_pool(name="scratch", bufs=4) as scpool,
        ):
            idx_t = cpool.tile([P, NBLK * K], i32)
            nc.sync.dma_start(idx_t[:], idx[:])
            acc = cpool.tile([P, NBLK], f32)

            for j in range(NBLK):
                g = gpool.tile([P, K, DIM], f32, tag="g")
                # One [128,1]-offset gather per role: the only offset-AP
                # shape this walrus generates correct descriptors for.
                for k in range(K):
                    nc.gpsimd.indirect_dma_start(
                        out=g[:, k, :],
                        out_offset=None,
                        in_=table[:],
                        in_offset=bass.IndirectOffsetOnAxis(
                            ap=idx_t[:, j * K + k : j * K + k + 1], axis=0
                        ),
                    )

                # sum of the 10 context rows -> [P, DIM]
                su = spool.tile([P, DIM], f32, tag="su")
                nc.vector.tensor_reduce(
                    out=su[:],
                    in_=g[:, 0:CTX, :].rearrange("p n d -> p d n"),
                    axis=mybir.AxisListType.X,
                    op=mybir.AluOpType.add,
                )

                # 11 fused dots: raw[:, n] = sum_d (±0.1 * v_row_n) * su
                # n=0 (pos sample) carries the minus sign so that the loss is
                # softplus(raw_n) uniformly for all n.
                raw = spool.tile([P, ND], f32, tag="raw")
                for n in range(ND):
                    so = scpool.tile([P, DIM], f32, tag="so")
                    nc.vector.scalar_tensor_tensor(
                        out=so[:],
                        in0=g[:, CTX + n, :],
                        scalar=(-1.0 if n == 0 else 1.0) / CTX,
                        in1=su[:],
                        op0=mybir.AluOpType.mult,
                        op1=mybir.AluOpType.mult,
                        accum_out=raw[:, n : n + 1],
                    )

                # clip to [-10, 10] in one fused op
                rc = spool.tile([P, ND], f32, tag="rc")
                nc.vector.tensor_scalar(
                    out=rc[:],
                    in0=raw[:],
                    scalar1=-10.0,
                    scalar2=10.0,
                    op0=mybir.AluOpType.max,
                    op1=mybir.AluOpType.min,
                )

                # softplus(x) = ln(1 + exp(x)); accumulate the 11 terms into
                # this block's accumulator column.
                ex = scpool.tile([P, ND], f32, tag="ex")
                nc.scalar.activation(
                    out=ex[:],
                    in_=rc[:],
                    func=mybir.ActivationFunctionType.Exp,
                )
                sp = scpool.tile([P, ND], f32, tag="sp")
                nc.scalar.activation(
                    out=sp[:],
                    in_=ex[:],
                    func=mybir.ActivationFunctionType.Ln,
                    bias=1.0,
                    accum_out=acc[:, j : j + 1],
                )

            nc.sync.dma_start(out[:], acc[:])

    _split_excess_waits(nc, mybir)
    _PROGRAM_CACHE[gather_bufs] = nc
    return nc


def _prep_inputs(pos_u, pos_v, neg_v, u_weight, v_weight):
    """Shard + repack host-side. Returns per-core input maps."""
    table = np.ascontiguousarray(
        np.concatenate(
            [np.asarray(u_weight, np.float32), np.asarray(v_weight, np.float32)],
            axis=0,
        )
    )
    pos_u = np.asarray(pos_u, np.int32)
    pos_v = np.asarray(pos_v, np.int32)
    neg_v = np.asarray(neg_v, np.int32)

    in_maps = []
    for c in range(N_CORES):
        s = slice(c * B_SHARD, (c + 1) * B_SHARD)
        ia = np.empty((B_SHARD, K), np.int32)
        ia[:, 0:CTX] = pos_u[s]
        ia[:, CTX] = pos_v[s] + VOCAB
        ia[:, CTX + 1 : K] = neg_v[s] + VOCAB
        idx_dram = np.ascontiguousarray(
            ia.reshape(NBLK, P, K).transpose(1, 0, 2).reshape(P, NBLK * K)
        )
        in_maps.append({"table": table, "idx": idx_dram})
    return in_maps


def _run(pos_u, pos_v, neg_v, u_weight, v_weight, trace=False):
    from concourse.bass_utils import run_bass_kernel_spmd

    nc = _build_program()
    in_maps = _prep_inputs(pos_u, pos_v, neg_v, u_weight, v_weight)
    res = run_bass_kernel_spmd(nc, in_maps, list(range(N_CORES)), trace=trace)
    total = 0.0
    for c in range(N_CORES):
        total += res.results[c]["out"].sum(dtype=np.float64)
    loss = np.array(total / B, dtype=np.float32)
    return loss, res


def kernel(pos_u, pos_v, neg_v, u_weight, v_weight):
    loss, _ = _run(pos_u, pos_v, neg_v, u_weight, v_weight, trace=False)
    return loss
